# revision 13
# baseline (speedup 1.0000x reference)
"""BlockSparseAttention TRN2 kernel: 8-core SPMD (batch x head-group sharding).

Core c handles batch b = c//4, heads [4g, 4g+4) with g = c%4.
Per (head, 128-row query tile i): attention is nonzero only on
  - window blocks: aligned 128-col blocks j in {i-1, i, i+1} (covers |q-k|<=64 band)
  - landmark columns (gathered, <=128 slots)
Device computes normalized attention values for exactly those regions plus the
out-projection partial; host assembles the full [B,H,S,S] attn tensor (zeros
elsewhere -- exact: masked entries underflow to 0.0 in fp32 softmax) and sums
the out-proj partials (the out_proj all-reduce).
"""
import sys, os
sys.path.insert(0, "/opt/trn_rl_repo")
import numpy as np
from contextlib import ExitStack

import concourse.bass as bass
import concourse.tile as tile
from concourse import bacc, mybir
from concourse.bass_utils import run_bass_kernel_spmd

F32 = mybir.dt.float32
F32R = mybir.dt.float32r
F16 = mybir.dt.float16

B, S, D = 2, 2048, 1024
H, HD = 16, 64
W2 = 64              # half window
SCALE = 1.0 / 8.0
NEG = -10000.0
NT = 16              # query tiles of 128
NLM = 128            # landmark slots (padded)
NPAIR = 8            # tile pairs

_cache = {}


def _valid_ks(i):
    ks = [k for k in (0, 1, 2) if 0 <= i - 1 + k < NT]
    return ks[0], len(ks)          # kf, NW


def build_nc():
    nc = bacc.Bacc("TRN2", target_bir_lowering=False, debug=False, num_devices=8)

    xT_d = nc.dram_tensor("xT", [D, S], F32, kind="ExternalInput")
    xTlm_d = nc.dram_tensor("xTlm", [D, NLM], F32, kind="ExternalInput")
    wT_d = nc.dram_tensor("wT", [D, 768], F32, kind="ExternalInput")
    bqk_d = nc.dram_tensor("bqk", [512], F32, kind="ExternalInput")
    bv_d = nc.dram_tensor("bv", [256], F32, kind="ExternalInput")
    woT_d = nc.dram_tensor("woT", [256, 1024], F32, kind="ExternalInput")
    bandQ_d = nc.dram_tensor("bandQ", [3, 128, 128], F32, kind="ExternalInput")
    lmq_d = nc.dram_tensor("lmq", [S], F32, kind="ExternalInput")
    lmqm_d = nc.dram_tensor("lmqm", [NT, NLM], F32, kind="ExternalInput")
    ident_d = nc.dram_tensor("ident", [128, 128], F32, kind="ExternalInput")

    attw_d = nc.dram_tensor("attn_w", [NPAIR, 128, 4, 2, 3, 128], F16, kind="ExternalOutput")
    attl_d = nc.dram_tensor("attn_lm", [NPAIR, 128, 4, 2, NLM], F16, kind="ExternalOutput")
    outp_d = nc.dram_tensor("outp", [S, 1024], F32, kind="ExternalOutput")

    with tile.TileContext(nc) as tc, ExitStack() as ctx:
        persist = ctx.enter_context(tc.tile_pool(name="persist", bufs=1))

        # persistent tiles
        qkT = [persist.tile([128, S], F32R, tag=f"qkT{m}", name=f"qkT{m}") for m in range(4)]  # q01,q23,k01,k23
        kTlm = [persist.tile([128, NLM], F32R, tag=f"kTlm{m}", name=f"kTlm{m}") for m in range(2)]
        Vn = persist.tile([128, NT, 256], F16)
        Vlm = persist.tile([128, 256], F16)
        maskC = persist.tile([128, NT, 512], F32)
        concT = [persist.tile([128, S], F32R, tag=f"concT{m}", name=f"concT{m}") for m in range(2)]
        woT_s = persist.tile([128, 2, 1024], F32R)
        ident_s = persist.tile([128, 128], F16)
        bandQ_s = persist.tile([128, 3, 128], F32)
        lmq_s = persist.tile([128, S], F32)
        bqk_s = persist.tile([128, 4], F32)
        bv_s = persist.tile([128, 256], F32)

        def bcast(ap, n_free_dims=None):
            a = ap if isinstance(ap, bass.AP) else ap.ap()
            return bass.AP(tensor=a.tensor, offset=a.offset, ap=[[0, 128]] + list(a.ap))

        nc.gpsimd.dma_start(out=ident_s, in_=ident_d[:, :])
        nc.sync.dma_start(out=bandQ_s, in_=bandQ_d.rearrange("k q c -> q k c"))
        nc.sync.dma_start(out=lmq_s, in_=bcast(lmq_d))
        nc.sync.dma_start(out=maskC[:, :, 384:512], in_=bcast(lmqm_d))
        nc.sync.dma_start(out=bqk_s, in_=bqk_d.rearrange("(g p) -> p g", p=128))
        nc.sync.dma_start(out=bv_s, in_=bcast(bv_d))
        nc.gpsimd.dma_start(out=woT_s, in_=woT_d.rearrange("(u p) o -> p u o", p=128))

        # ---------------- phase 1: projections ----------------
        with tc.tile_pool(name="p1", bufs=2) as p1, \
             tc.tile_pool(name="p1w", bufs=1) as p1w, \
             tc.tile_pool(name="pp1", bufs=2, space="PSUM") as pp1, \
             tc.tile_pool(name="pp1v", bufs=2, space="PSUM") as pp1v:
            wt = p1w.tile([128, 8, 768], F32R)
            nc.gpsimd.dma_start(out=wt, in_=wT_d.rearrange("(c p) m -> p c m", p=128))
            xlm = p1w.tile([128, 8, NLM], F32R)
            nc.gpsimd.dma_start(out=xlm, in_=xTlm_d.rearrange("(c p) l -> p c l", p=128))

            for sc in range(4):
                xt = p1.tile([128, 8, 512], F32R, tag="xt")
                nc.gpsimd.dma_start(
                    out=xt, in_=xT_d[:, 512 * sc:512 * (sc + 1)].rearrange(
                        "(c p) s -> p c s", p=128))
                for grp in range(4):
                    ps = pp1.tile([128, 512], F32, tag="ps1")
                    for c in range(8):
                        nc.tensor.matmul(ps, wt[:, c, 128 * grp:128 * (grp + 1)],
                                         xt[:, c, :], start=(c == 0), stop=(c == 7))
                    nc.scalar.activation(
                        out=qkT[grp][:, 512 * sc:512 * (sc + 1)], in_=ps,
                        func=mybir.ActivationFunctionType.Identity,
                        bias=bqk_s[:, grp:grp + 1])
                for u in range(4):
                    psv = pp1v.tile([128, 256], F32, tag="psv")
                    for c in range(8):
                        nc.tensor.matmul(psv, xt[:, c, 128 * u:128 * (u + 1)],
                                         wt[:, c, 512:768], start=(c == 0), stop=(c == 7))
                    nc.vector.tensor_add(Vn[:, 4 * sc + u, :], psv, bv_s)

            for pr in range(2):
                ps = pp1v.tile([128, NLM], F32, tag="psv")
                for c in range(8):
                    nc.tensor.matmul(ps, wt[:, c, 256 + 128 * pr:384 + 128 * pr],
                                     xlm[:, c, :], start=(c == 0), stop=(c == 7))
                nc.vector.tensor_scalar_add(kTlm[pr], ps, bqk_s[:, 2 + pr:3 + pr])
            psv = pp1v.tile([128, 256], F32, tag="psv")
            for c in range(8):
                nc.tensor.matmul(psv, xlm[:, c, :], wt[:, c, 512:768],
                                 start=(c == 0), stop=(c == 7))
            nc.vector.tensor_add(Vlm, psv, bv_s)

        # union masks: maskC[i, 128s:128s+128] = max(band_{kf+s}, lm_additive[block j])
        for i in range(NT):
            kf, NW = _valid_ks(i)
            for s in range(NW):
                k = kf + s
                j = i - 1 + k
                nc.vector.tensor_max(maskC[:, i, 128 * s:128 * (s + 1)],
                                     bandQ_s[:, k, :],
                                     lmq_s[:, 128 * j:128 * (j + 1)])
            if NW == 2:
                nc.vector.memset(maskC[:, i, 256:384], NEG)

        # ---------------- phase 2: attention ----------------
        with tc.tile_pool(name="p2", bufs=2) as p2, \
             tc.tile_pool(name="p2b", bufs=3) as p2b, \
             tc.tile_pool(name="psA", bufs=2, space="PSUM") as psA, \
             tc.tile_pool(name="psT", bufs=2, space="PSUM") as psTp, \
             tc.tile_pool(name="psO", bufs=1, space="PSUM") as psOp, \
             tc.tile_pool(name="psF", bufs=1, space="PSUM") as psFp:
            for t in range(NPAIR):
                jlo = max(0, 2 * t - 1)            # lowest block index this pair
                for h in range(4):
                    pr, sub = h // 2, h % 2
                    lo, hi = 64 * sub, 64 * sub + 64
                    psT = psTp.tile([128, 5, 256], F16, tag="psT")
                    attn_n = p2b.tile([128, 1024], F16, tag="attn_n")
                    for ii in (2 * t, 2 * t + 1):
                        kf, NW = _valid_ks(ii)
                        qh = ii - 2 * t
                        qs = qkT[pr][lo:hi, 128 * ii:128 * (ii + 1)]
                        ps_q = psA.tile([128, 512], F32, tag="ps_q")
                        nc.tensor.matmul(
                            ps_q[:, 0:NW * 128], qs,
                            qkT[2 + pr][lo:hi, 128 * (ii - 1 + kf):128 * (ii - 1 + kf + NW)],
                            start=True, stop=True)
                        nc.tensor.matmul(ps_q[:, 384:512], qs, kTlm[pr][lo:hi, :],
                                         start=True, stop=True)
                        stage = p2.tile([128, 512], F32, tag="stage")
                        nc.vector.tensor_add(stage, ps_q, maskC[:, ii, :])
                        if NW == 2:
                            nc.vector.memset(stage[:, 256:384], NEG)
                        attn_u = p2.tile([128, 512], F16, tag="attn_u")
                        Zq = p2b.tile([128, 1], F32, tag="Zq")
                        nc.scalar.activation(out=attn_u, in_=stage,
                                             func=mybir.ActivationFunctionType.Exp,
                                             accum_out=Zq)
                        rz = p2b.tile([128, 1], F32, tag="rz")
                        nc.vector.reciprocal(rz, Zq)
                        nc.vector.tensor_scalar_mul(
                            attn_n[:, 512 * qh:512 * (qh + 1)], attn_u, rz)
                        for k in range(kf, kf + NW):
                            j = ii - 1 + k
                            nc.tensor.transpose(
                                psT[:, j - jlo, 128 * qh:128 * (qh + 1)],
                                attn_n[:, 512 * qh + 128 * (k - kf):512 * qh + 128 * (k - kf + 1)],
                                ident_s)
                        nc.tensor.transpose(psT[:, 4, 128 * qh:128 * (qh + 1)],
                                            attn_n[:, 512 * qh + 384:512 * qh + 512],
                                            ident_s)
                    nc.sync.dma_start(
                        out=attw_d[t, :, h, :, :, :].rearrange("q i k c -> q i (k c)"),
                        in_=bass.AP(tensor=attn_n.tensor, offset=attn_n.offset,
                                    ap=list(attn_n.ap[:1]) + [[512, 2], [1, 384]]))
                    nc.sync.dma_start(
                        out=attl_d[t, :, h, :, :],
                        in_=bass.AP(tensor=attn_n.tensor, offset=attn_n.offset + 384,
                                    ap=list(attn_n.ap[:1]) + [[512, 2], [1, 128]]))
                    attnT = p2.tile([128, 5, 256], F16, tag="attnT")
                    nc.vector.tensor_copy(attnT[:, 0:3, :], psT[:, 0:3, :])
                    nc.scalar.copy(attnT[:, 3:5, :], psT[:, 3:5, :])
                    # attn @ V  (transposed out: [64, 256])
                    ps_o = psOp.tile([64, 256], F32, tag="ps_o")
                    mms = [(2 * t, 0, 256), (2 * t + 1, 0, 256)]
                    if t > 0:
                        mms.append((2 * t - 1, 0, 128))
                    if t < NPAIR - 1:
                        mms.append((2 * t + 2, 128, 256))
                    for n, (j, c0, c1) in enumerate(mms):
                        nc.tensor.matmul(ps_o[:, c0:c1], Vn[:, j, 64 * h:64 * (h + 1)],
                                         attnT[:, j - jlo, c0:c1],
                                         start=(n == 0), stop=False)
                    nc.tensor.matmul(ps_o, Vlm[:, 64 * h:64 * (h + 1)], attnT[:, 4, :],
                                     start=False, stop=True)
                    nc.vector.tensor_copy(concT[pr][lo:hi, 256 * t:256 * (t + 1)], ps_o)
                # out-projection for the pair's two q-tiles
                outp_s = p2.tile([128, 2048], F32, tag="outp_s")
                for qh in range(2):
                    ii = 2 * t + qh
                    for half in range(2):
                        ps_f = psFp.tile([128, 512], F32, tag="ps_f")
                        for p2i in range(2):
                            nc.tensor.matmul(
                                ps_f, concT[p2i][:, 128 * ii:128 * (ii + 1)],
                                woT_s[:, p2i, 512 * half:512 * (half + 1)],
                                start=(p2i == 0), stop=(p2i == 1))
                        dst = outp_s[:, 1024 * qh + 512 * half:1024 * qh + 512 * (half + 1)]
                        if half == 0:
                            nc.vector.tensor_copy(dst, ps_f)
                        else:
                            nc.scalar.copy(dst, ps_f)
                nc.sync.dma_start(
                    out=outp_d.rearrange("(t i q) o -> t q i o", q=128, i=2)[t],
                    in_=outp_s.rearrange("q (i o) -> q i o", i=2))

    nc.compile()
    return nc


def kernel(x, landmark_mask, qkv_w, qkv_b, out_w, out_b):
    x = np.asarray(x, np.float32)
    landmark_mask = np.asarray(landmark_mask)
    qkv_w = np.asarray(qkv_w, np.float32)
    qkv_b = np.asarray(qkv_b, np.float32)
    out_w = np.asarray(out_w, np.float32)
    out_b = np.asarray(out_b, np.float32)

    if "nc" not in _cache:
        _cache["nc"] = build_nc()
    nc = _cache["nc"]

    # static aux
    q = np.arange(128)[:, None]
    c = np.arange(128)[None, :]
    bandQ = np.stack([np.where(np.abs(q - c + 128 * (1 - k)) <= W2, 0.0, NEG)
                      for k in range(3)]).astype(np.float32)
    ident = np.eye(128, dtype=np.float32)

    lm_idx, n_lm = [], []
    per_b = {}
    for b in range(B):
        idx = np.nonzero(np.asarray(landmark_mask[b, 0]))[0].astype(np.int64)
        assert idx.size <= NLM, f"too many landmarks ({idx.size})"
        n = idx.size
        pad = np.zeros(NLM, np.int64)
        pad[:n] = idx
        lm_idx.append(pad)
        n_lm.append(n)
        xb = x[b]                                   # [S, D]
        xTlm = np.zeros((D, NLM), np.float32)
        xTlm[:, :n] = xb[idx].T
        lmq = np.where(np.asarray(landmark_mask[b, 0]), 0.0, NEG).astype(np.float32)
        lmqm = np.full((NT, NLM), NEG, np.float32)
        for i in range(NT):
            lo_r, hi_r = max(0, 128 * (i - 1)), min(S, 128 * (i + 2))
            for l in range(n):
                if not (lo_r <= idx[l] < hi_r):
                    lmqm[i, l] = 0.0
        per_b[b] = dict(xT=np.ascontiguousarray(xb.T), xTlm=xTlm, lmq=lmq, lmqm=lmqm)

    in_maps = []
    for core in range(8):
        b, g = core // 4, core % 4
        hs = [4 * g + j for j in range(4)]
        wq = np.concatenate([qkv_w[64 * h:64 * h + 64] for h in hs]) * SCALE
        wk = np.concatenate([qkv_w[D + 64 * h:D + 64 * h + 64] for h in hs])
        wv = np.concatenate([qkv_w[2 * D + 64 * h:2 * D + 64 * h + 64] for h in hs])
        wT = np.ascontiguousarray(np.concatenate([wq, wk, wv]).T)   # [1024, 768]
        bq = np.concatenate([qkv_b[64 * h:64 * h + 64] for h in hs]) * SCALE
        bk = np.concatenate([qkv_b[D + 64 * h:D + 64 * h + 64] for h in hs])
        bv_ = np.concatenate([qkv_b[2 * D + 64 * h:2 * D + 64 * h + 64] for h in hs])
        bqk = np.concatenate([bq, bk]).astype(np.float32)           # [512]
        woT = np.ascontiguousarray(out_w[:, 256 * g:256 * (g + 1)].T)  # [256, 1024]
        in_maps.append(dict(per_b[b],
                            wT=wT, bqk=bqk, bv=bv_.astype(np.float32), woT=woT,
                            bandQ=bandQ, ident=ident))

    _cache["last_in_maps"] = in_maps
    res = run_bass_kernel_spmd(nc, in_maps, list(range(8)))
    results = res.results

    attn = np.zeros((B, H, S, S), np.float32)
    out = np.empty((B, S, D), np.float32)
    for b in range(B):
        acc = np.zeros((S, D), np.float32)
        n = n_lm[b]
        cols = lm_idx[b][:n]
        for g in range(4):
            r = results[4 * b + g]
            acc += r["outp"]
            aw, al = r["attn_w"], r["attn_lm"]
            for hh in range(4):
                Hg = 4 * g + hh
                if n:
                    attn[b, Hg][:, cols] = \
                        al[:, :, hh].transpose(0, 2, 1, 3).reshape(S, NLM)[:, :n]
                awh = aw[:, :, hh].transpose(0, 2, 1, 3, 4).reshape(NT, 128, 3, 128)
                for i in range(NT):
                    kf, NW = _valid_ks(i)
                    for s in range(NW):
                        j = i - 1 + kf + s
                        attn[b, Hg, 128 * i:128 * (i + 1), 128 * j:128 * (j + 1)] = \
                            awh[i, :, s, :]
        out[b] = acc + out_b
    return out, attn


# revision 14
# speedup vs baseline: 1.1822x; 1.1822x over previous
"""BlockSparseAttention TRN2 kernel: 8-core SPMD (batch x head-group sharding).

Core c handles batch b = c//4, heads [4g, 4g+4) with g = c%4.
Per (head, 128-row query tile i): attention is nonzero only on
  - window blocks: aligned 128-col blocks j in {i-1, i, i+1} (covers |q-k|<=64 band)
  - landmark columns (gathered, <=128 slots)
Device computes normalized attention values for exactly those regions plus the
out-projection partial; host assembles the full [B,H,S,S] attn tensor (zeros
elsewhere -- exact: masked entries underflow to 0.0 in fp32 softmax) and sums
the out-proj partials (the out_proj all-reduce).
"""
import sys, os
sys.path.insert(0, "/opt/trn_rl_repo")
import numpy as np
from contextlib import ExitStack

import concourse.bass as bass
import concourse.tile as tile
from concourse import bacc, mybir
from concourse.bass_utils import run_bass_kernel_spmd

F32 = mybir.dt.float32
F32R = mybir.dt.float32r
F16 = mybir.dt.float16

B, S, D = 2, 2048, 1024
H, HD = 16, 64
W2 = 64              # half window
SCALE = 1.0 / 8.0
NEG = -10000.0
NT = 16              # query tiles of 128
NLM = 128            # landmark slots (padded)
NPAIR = 8            # tile pairs

_cache = {}


def _valid_ks(i):
    ks = [k for k in (0, 1, 2) if 0 <= i - 1 + k < NT]
    return ks[0], len(ks)          # kf, NW


def build_nc():
    nc = bacc.Bacc("TRN2", target_bir_lowering=False, debug=False, num_devices=8)

    xT_d = nc.dram_tensor("xT", [D, S], F32, kind="ExternalInput")
    xTlm_d = nc.dram_tensor("xTlm", [D, NLM], F32, kind="ExternalInput")
    wT_d = nc.dram_tensor("wT", [D, 768], F32, kind="ExternalInput")
    bqk_d = nc.dram_tensor("bqk", [512], F32, kind="ExternalInput")
    bv_d = nc.dram_tensor("bv", [256], F32, kind="ExternalInput")
    woT_d = nc.dram_tensor("woT", [256, 1024], F32, kind="ExternalInput")
    bandQ_d = nc.dram_tensor("bandQ", [3, 128, 128], F32, kind="ExternalInput")
    lmq_d = nc.dram_tensor("lmq", [S], F32, kind="ExternalInput")
    lmqm_d = nc.dram_tensor("lmqm", [NT, NLM], F32, kind="ExternalInput")
    ident_d = nc.dram_tensor("ident", [128, 128], F32, kind="ExternalInput")

    attw_d = nc.dram_tensor("attn_w", [NPAIR, 128, 4, 2, 3, 128], F16, kind="ExternalOutput")
    attl_d = nc.dram_tensor("attn_lm", [NPAIR, 128, 4, 2, NLM], F16, kind="ExternalOutput")
    outp_d = nc.dram_tensor("outp", [S, 1024], F32, kind="ExternalOutput")

    with tile.TileContext(nc) as tc, ExitStack() as ctx:
        persist = ctx.enter_context(tc.tile_pool(name="persist", bufs=1))

        # persistent tiles
        qkT = [persist.tile([128, S], F32R, tag=f"qkT{m}", name=f"qkT{m}") for m in range(4)]  # q01,q23,k01,k23
        kTlm = [persist.tile([128, NLM], F32R, tag=f"kTlm{m}", name=f"kTlm{m}") for m in range(2)]
        Vn = persist.tile([128, NT, 256], F16)
        Vlm = persist.tile([128, 256], F16)
        maskC = persist.tile([128, NT, 512], F32)
        concT = [persist.tile([128, S], F32R, tag=f"concT{m}", name=f"concT{m}") for m in range(2)]
        woT_s = persist.tile([128, 2, 1024], F32R)
        ident_s = persist.tile([128, 128], F16)
        bandQ_s = persist.tile([128, 3, 128], F32)
        lmq_s = persist.tile([128, S], F32)
        bqk_s = persist.tile([128, 4], F32)
        bv_s = persist.tile([128, 256], F32)

        def bcast(ap, n_free_dims=None):
            a = ap if isinstance(ap, bass.AP) else ap.ap()
            return bass.AP(tensor=a.tensor, offset=a.offset, ap=[[0, 128]] + list(a.ap))

        nc.gpsimd.dma_start(out=ident_s, in_=ident_d[:, :])
        nc.sync.dma_start(out=bandQ_s, in_=bandQ_d.rearrange("k q c -> q k c"))
        nc.sync.dma_start(out=lmq_s, in_=bcast(lmq_d))
        nc.sync.dma_start(out=maskC[:, :, 384:512], in_=bcast(lmqm_d))
        nc.sync.dma_start(out=bqk_s, in_=bqk_d.rearrange("(g p) -> p g", p=128))
        nc.sync.dma_start(out=bv_s, in_=bcast(bv_d))
        nc.gpsimd.dma_start(out=woT_s, in_=woT_d.rearrange("(u p) o -> p u o", p=128))

        # ---------------- phase 1: projections ----------------
        with tc.tile_pool(name="p1", bufs=2) as p1, \
             tc.tile_pool(name="p1w", bufs=1) as p1w, \
             tc.tile_pool(name="pp1", bufs=2, space="PSUM") as pp1, \
             tc.tile_pool(name="pp1v", bufs=2, space="PSUM") as pp1v:
            wt = p1w.tile([128, 8, 768], F32R)
            nc.gpsimd.dma_start(out=wt, in_=wT_d.rearrange("(c p) m -> p c m", p=128))
            xlm = p1w.tile([128, 8, NLM], F32R)
            nc.gpsimd.dma_start(out=xlm, in_=xTlm_d.rearrange("(c p) l -> p c l", p=128))

            for sc in range(4):
                xt = p1.tile([128, 8, 512], F32R, tag="xt")
                nc.gpsimd.dma_start(
                    out=xt, in_=xT_d[:, 512 * sc:512 * (sc + 1)].rearrange(
                        "(c p) s -> p c s", p=128))
                for grp in range(4):
                    ps = pp1.tile([128, 512], F32, tag="ps1")
                    for c in range(8):
                        nc.tensor.matmul(ps, wt[:, c, 128 * grp:128 * (grp + 1)],
                                         xt[:, c, :], start=(c == 0), stop=(c == 7))
                    nc.scalar.activation(
                        out=qkT[grp][:, 512 * sc:512 * (sc + 1)], in_=ps,
                        func=mybir.ActivationFunctionType.Identity,
                        bias=bqk_s[:, grp:grp + 1])
                for u in range(4):
                    psv = pp1v.tile([128, 256], F32, tag="psv")
                    for c in range(8):
                        nc.tensor.matmul(psv, xt[:, c, 128 * u:128 * (u + 1)],
                                         wt[:, c, 512:768], start=(c == 0), stop=(c == 7))
                    nc.vector.tensor_add(Vn[:, 4 * sc + u, :], psv, bv_s)

            for pr in range(2):
                ps = pp1v.tile([128, NLM], F32, tag="psv")
                for c in range(8):
                    nc.tensor.matmul(ps, wt[:, c, 256 + 128 * pr:384 + 128 * pr],
                                     xlm[:, c, :], start=(c == 0), stop=(c == 7))
                nc.vector.tensor_scalar_add(kTlm[pr], ps, bqk_s[:, 2 + pr:3 + pr])
            psv = pp1v.tile([128, 256], F32, tag="psv")
            for c in range(8):
                nc.tensor.matmul(psv, xlm[:, c, :], wt[:, c, 512:768],
                                 start=(c == 0), stop=(c == 7))
            nc.vector.tensor_add(Vlm, psv, bv_s)

        # union masks: maskC[i, 128s:128s+128] = max(band_{kf+s}, lm_additive[block j])
        for i in range(NT):
            kf, NW = _valid_ks(i)
            for s in range(NW):
                k = kf + s
                j = i - 1 + k
                nc.vector.tensor_max(maskC[:, i, 128 * s:128 * (s + 1)],
                                     bandQ_s[:, k, :],
                                     lmq_s[:, 128 * j:128 * (j + 1)])
            if NW == 2:
                nc.vector.memset(maskC[:, i, 256:384], NEG)

        # ---------------- phase 2: attention ----------------
        with tc.tile_pool(name="p2", bufs=2) as p2, \
             tc.tile_pool(name="p2b", bufs=3) as p2b, \
             tc.tile_pool(name="psA", bufs=2, space="PSUM") as psA, \
             tc.tile_pool(name="psT", bufs=1, space="PSUM") as psTp, \
             tc.tile_pool(name="psO", bufs=1, space="PSUM") as psOp, \
             tc.tile_pool(name="psF", bufs=1, space="PSUM") as psFp:
            for t in range(NPAIR):
                jlo = max(0, 2 * t - 1)            # lowest block index this pair
                for h in range(4):
                    pr, sub = h // 2, h % 2
                    lo, hi = 64 * sub, 64 * sub + 64
                    psT = psTp.tile([128, 5, 256], F16, tag="psT")
                    attn_n = p2b.tile([128, 1024], F16, tag="attn_n")
                    for ii in (2 * t, 2 * t + 1):
                        kf, NW = _valid_ks(ii)
                        qh = ii - 2 * t
                        qs = qkT[pr][lo:hi, 128 * ii:128 * (ii + 1)]
                        ps_q = psA.tile([128, 512], F32, tag="ps_q")
                        nc.tensor.matmul(
                            ps_q[:, 0:NW * 128], qs,
                            qkT[2 + pr][lo:hi, 128 * (ii - 1 + kf):128 * (ii - 1 + kf + NW)],
                            start=True, stop=True)
                        nc.tensor.matmul(ps_q[:, 384:512], qs, kTlm[pr][lo:hi, :],
                                         start=True, stop=True)
                        stage = p2.tile([128, 512], F32, tag="stage")
                        nc.vector.tensor_add(stage, ps_q, maskC[:, ii, :])
                        if NW == 2:
                            nc.vector.memset(stage[:, 256:384], NEG)
                        attn_u = p2.tile([128, 512], F16, tag="attn_u")
                        Zq = p2b.tile([128, 1], F32, tag="Zq")
                        nc.scalar.activation(out=attn_u, in_=stage,
                                             func=mybir.ActivationFunctionType.Exp,
                                             accum_out=Zq)
                        rz = p2b.tile([128, 1], F32, tag="rz")
                        nc.vector.reciprocal(rz, Zq)
                        nc.vector.tensor_scalar_mul(
                            attn_n[:, 512 * qh:512 * (qh + 1)], attn_u, rz)
                        for k in range(kf, kf + NW):
                            j = ii - 1 + k
                            nc.tensor.transpose(
                                psT[:, j - jlo, 128 * qh:128 * (qh + 1)],
                                attn_n[:, 512 * qh + 128 * (k - kf):512 * qh + 128 * (k - kf + 1)],
                                ident_s)
                        nc.tensor.transpose(psT[:, 4, 128 * qh:128 * (qh + 1)],
                                            attn_n[:, 512 * qh + 384:512 * qh + 512],
                                            ident_s)
                    nc.sync.dma_start(
                        out=attw_d[t, :, h, :, :, :].rearrange("q i k c -> q i (k c)"),
                        in_=bass.AP(tensor=attn_n.tensor, offset=attn_n.offset,
                                    ap=list(attn_n.ap[:1]) + [[512, 2], [1, 384]]))
                    nc.sync.dma_start(
                        out=attl_d[t, :, h, :, :],
                        in_=bass.AP(tensor=attn_n.tensor, offset=attn_n.offset + 384,
                                    ap=list(attn_n.ap[:1]) + [[512, 2], [1, 128]]))
                    attnT = p2.tile([128, 5, 256], F16, tag="attnT")
                    nc.vector.tensor_copy(attnT[:, 0:3, :], psT[:, 0:3, :])
                    nc.scalar.copy(attnT[:, 3:5, :], psT[:, 3:5, :])
                    # attn @ V  (transposed out: [64, 256])
                    ps_o = psOp.tile([64, 256], F32, tag="ps_o")
                    mms = [(2 * t, 0, 256), (2 * t + 1, 0, 256)]
                    if t > 0:
                        mms.append((2 * t - 1, 0, 128))
                    if t < NPAIR - 1:
                        mms.append((2 * t + 2, 128, 256))
                    for n, (j, c0, c1) in enumerate(mms):
                        nc.tensor.matmul(ps_o[:, c0:c1], Vn[:, j, 64 * h:64 * (h + 1)],
                                         attnT[:, j - jlo, c0:c1],
                                         start=(n == 0), stop=False)
                    nc.tensor.matmul(ps_o, Vlm[:, 64 * h:64 * (h + 1)], attnT[:, 4, :],
                                     start=False, stop=True)
                    nc.vector.tensor_copy(concT[pr][lo:hi, 256 * t:256 * (t + 1)], ps_o)
                # out-projection for the pair's two q-tiles
                outp_s = p2.tile([128, 2048], F32, tag="outp_s")
                for qh in range(2):
                    ii = 2 * t + qh
                    for half in range(2):
                        ps_f = psFp.tile([128, 512], F32, tag="ps_f")
                        for p2i in range(2):
                            nc.tensor.matmul(
                                ps_f, concT[p2i][:, 128 * ii:128 * (ii + 1)],
                                woT_s[:, p2i, 512 * half:512 * (half + 1)],
                                start=(p2i == 0), stop=(p2i == 1))
                        dst = outp_s[:, 1024 * qh + 512 * half:1024 * qh + 512 * (half + 1)]
                        if half == 0:
                            nc.vector.tensor_copy(dst, ps_f)
                        else:
                            nc.scalar.copy(dst, ps_f)
                nc.sync.dma_start(
                    out=outp_d.rearrange("(t i q) o -> t q i o", q=128, i=2)[t],
                    in_=outp_s.rearrange("q (i o) -> q i o", i=2))

    nc.compile()
    return nc


def kernel(x, landmark_mask, qkv_w, qkv_b, out_w, out_b):
    x = np.asarray(x, np.float32)
    landmark_mask = np.asarray(landmark_mask)
    qkv_w = np.asarray(qkv_w, np.float32)
    qkv_b = np.asarray(qkv_b, np.float32)
    out_w = np.asarray(out_w, np.float32)
    out_b = np.asarray(out_b, np.float32)

    if "nc" not in _cache:
        _cache["nc"] = build_nc()
    nc = _cache["nc"]

    # static aux
    q = np.arange(128)[:, None]
    c = np.arange(128)[None, :]
    bandQ = np.stack([np.where(np.abs(q - c + 128 * (1 - k)) <= W2, 0.0, NEG)
                      for k in range(3)]).astype(np.float32)
    ident = np.eye(128, dtype=np.float32)

    lm_idx, n_lm = [], []
    per_b = {}
    for b in range(B):
        idx = np.nonzero(np.asarray(landmark_mask[b, 0]))[0].astype(np.int64)
        assert idx.size <= NLM, f"too many landmarks ({idx.size})"
        n = idx.size
        pad = np.zeros(NLM, np.int64)
        pad[:n] = idx
        lm_idx.append(pad)
        n_lm.append(n)
        xb = x[b]                                   # [S, D]
        xTlm = np.zeros((D, NLM), np.float32)
        xTlm[:, :n] = xb[idx].T
        lmq = np.where(np.asarray(landmark_mask[b, 0]), 0.0, NEG).astype(np.float32)
        lmqm = np.full((NT, NLM), NEG, np.float32)
        for i in range(NT):
            lo_r, hi_r = max(0, 128 * (i - 1)), min(S, 128 * (i + 2))
            for l in range(n):
                if not (lo_r <= idx[l] < hi_r):
                    lmqm[i, l] = 0.0
        per_b[b] = dict(xT=np.ascontiguousarray(xb.T), xTlm=xTlm, lmq=lmq, lmqm=lmqm)

    in_maps = []
    for core in range(8):
        b, g = core // 4, core % 4
        hs = [4 * g + j for j in range(4)]
        wq = np.concatenate([qkv_w[64 * h:64 * h + 64] for h in hs]) * SCALE
        wk = np.concatenate([qkv_w[D + 64 * h:D + 64 * h + 64] for h in hs])
        wv = np.concatenate([qkv_w[2 * D + 64 * h:2 * D + 64 * h + 64] for h in hs])
        wT = np.ascontiguousarray(np.concatenate([wq, wk, wv]).T)   # [1024, 768]
        bq = np.concatenate([qkv_b[64 * h:64 * h + 64] for h in hs]) * SCALE
        bk = np.concatenate([qkv_b[D + 64 * h:D + 64 * h + 64] for h in hs])
        bv_ = np.concatenate([qkv_b[2 * D + 64 * h:2 * D + 64 * h + 64] for h in hs])
        bqk = np.concatenate([bq, bk]).astype(np.float32)           # [512]
        woT = np.ascontiguousarray(out_w[:, 256 * g:256 * (g + 1)].T)  # [256, 1024]
        in_maps.append(dict(per_b[b],
                            wT=wT, bqk=bqk, bv=bv_.astype(np.float32), woT=woT,
                            bandQ=bandQ, ident=ident))

    _cache["last_in_maps"] = in_maps
    res = run_bass_kernel_spmd(nc, in_maps, list(range(8)))
    results = res.results

    attn = np.zeros((B, H, S, S), np.float32)
    out = np.empty((B, S, D), np.float32)
    for b in range(B):
        acc = np.zeros((S, D), np.float32)
        n = n_lm[b]
        cols = lm_idx[b][:n]
        for g in range(4):
            r = results[4 * b + g]
            acc += r["outp"]
            aw, al = r["attn_w"], r["attn_lm"]
            for hh in range(4):
                Hg = 4 * g + hh
                if n:
                    attn[b, Hg][:, cols] = \
                        al[:, :, hh].transpose(0, 2, 1, 3).reshape(S, NLM)[:, :n]
                awh = aw[:, :, hh].transpose(0, 2, 1, 3, 4).reshape(NT, 128, 3, 128)
                for i in range(NT):
                    kf, NW = _valid_ks(i)
                    for s in range(NW):
                        j = i - 1 + kf + s
                        attn[b, Hg, 128 * i:128 * (i + 1), 128 * j:128 * (j + 1)] = \
                            awh[i, :, s, :]
        out[b] = acc + out_b
    return out, attn
